# revision 1
# baseline (speedup 1.0000x reference)
"""Distributed Bass kernel for nn_Attention (B=2,T=2048,E=1024,H=16) on 8 trn2 cores.

Sharding: core c = b*4+g handles batch b = c//4, heads 4g..4g+3 for attention
(tensor-parallel on qkv columns), then FFN data-parallel on t-rows g*512..(g+1)*512
after an intra-group AllToAll that redistributes attention output from
head-sharded to t-sharded.

All activations live in transposed layout on chip: [feature partitions, t free],
so every matmul contracts on the partition axis with zero on-chip transposes.
Host pre-transposes/casts x and weights (bf16), and post-transposes the output.

Causal structure: the reference multiplies raw scores by a lower-triangular 0/1
mask BEFORE softmax (masked logits -> exp(0)=1, not 0). So for key blocks
strictly above the diagonal P==1 exactly; their contribution
(suffix-sums of V and a count for the denominator) is injected with one extra
matmul per chunk using a host-provided block-indicator constant. Only the
lower-triangular + diagonal score blocks are computed/exp'd.
"""

import numpy as np
import ml_dtypes

BF16 = ml_dtypes.bfloat16

B, T, E, H = 2, 2048, 1024, 16
DH = 64          # head dim
HPC = 4          # heads per core
NCORES = 8
TQ = T // 4      # FFN t-rows per core (512)
NT = T // 128    # 16 t-blocks
NE = E // 128    # 8 E-tiles
CH = 512         # q-chunk size
NCH = T // CH    # 4 chunks

_NC_CACHE = {}


def _build_nc():
    import concourse.bass as bass
    import concourse.mybir as mybir
    import concourse.tile as tile
    from concourse import bacc

    fp32 = mybir.dt.float32
    bf16 = mybir.dt.bfloat16
    Exp = mybir.ActivationFunctionType.Exp
    Relu = mybir.ActivationFunctionType.Relu
    add = mybir.AluOpType.add
    mult = mybir.AluOpType.mult

    nc = bacc.Bacc(None, target_bir_lowering=False)

    # ---- parameters (per-core shards staged by host) ----
    xt_p = nc.declare_dram_parameter("xt", [E, T], bf16, isOutput=False)       # x[b].T
    wqk_p = nc.declare_dram_parameter("wqk", [E, 512], bf16, isOutput=False)   # [Qp0|Kp0|Qp1|Kp1]
    wv_p = nc.declare_dram_parameter("wv", [E, 256], bf16, isOutput=False)     # [v0|v1|v2|v3]
    bqk_p = nc.declare_dram_parameter("bqk", [128, 4], fp32, isOutput=False)
    c1_p = nc.declare_dram_parameter("c1t", [128, 8], fp32, isOutput=False)    # W1.T@bv + b1
    b2_p = nc.declare_dram_parameter("b2t", [128, 8], fp32, isOutput=False)
    w1_p = nc.declare_dram_parameter("w1", [E, E], bf16, isOutput=False)
    w2_p = nc.declare_dram_parameter("w2", [E, E], bf16, isOutput=False)
    mask_p = nc.declare_dram_parameter("maskc", [128, 128], mybir.dt.uint8, isOutput=False)
    ones_p = nc.declare_dram_parameter("onesc", [128, 128], bf16, isOutput=False)
    ind_p = nc.declare_dram_parameter("indc", [16, T], bf16, isOutput=False)
    gidx_p = nc.declare_dram_parameter("gidx", [128, 8], mybir.dt.int32, isOutput=False)
    selb_p = nc.declare_dram_parameter("selb", [2, 128], bf16, isOutput=False)
    sufc_p = nc.declare_dram_parameter("sufc", [128, 256], bf16, isOutput=False)
    out_p = nc.declare_dram_parameter("out", [E, TQ], fp32, isOutput=True)     # final out.T slice

    with tile.TileContext(nc) as tc:
        with (
            tc.tile_pool(name="const", bufs=1) as cpool,
            tc.tile_pool(name="wts", bufs=1) as wpool,
            tc.tile_pool(name="xt", bufs=1) as xpool,
            tc.tile_pool(name="qk", bufs=1) as qkpool,
            tc.tile_pool(name="vaug", bufs=1) as vpool,
            tc.tile_pool(name="attnT", bufs=1) as apool,
            tc.tile_pool(name="vsuf", bufs=1) as vspool,
            tc.tile_pool(name="ptile", bufs=6) as ppool,
            tc.tile_pool(name="rb", bufs=3) as rpool,
            tc.tile_pool(name="ffn", bufs=1) as fpool,
            tc.tile_pool(name="dram", bufs=1, space="DRAM") as dpool,
            tc.tile_pool(name="ps", bufs=2, space="PSUM") as pspool,
        ):
            # ---- load constants & weights ----
            maskc = cpool.tile([128, 128], mybir.dt.uint8, tag="maskc", name="maskc")
            nc.sync.dma_start(out=maskc[:, :], in_=mask_p[:, :])
            onesc = cpool.tile([128, 128], bf16, tag="onesc", name="onesc")
            nc.sync.dma_start(out=onesc[:, :], in_=ones_p[:, :])
            indc = cpool.tile([16, T], bf16, tag="indc", name="indc")
            nc.sync.dma_start(out=indc[:, :], in_=ind_p[:, :])
            bqk = cpool.tile([128, 4], fp32, tag="bqk", name="bqk")
            nc.sync.dma_start(out=bqk[:, :], in_=bqk_p[:, :])
            c1t = cpool.tile([128, 8], fp32, tag="c1t", name="c1t")
            nc.sync.dma_start(out=c1t[:, :], in_=c1_p[:, :])
            b2t = cpool.tile([128, 8], fp32, tag="b2t", name="b2t")
            nc.sync.dma_start(out=b2t[:, :], in_=b2_p[:, :])
            selb = cpool.tile([2, 128], bf16, tag="selb", name="selb")
            nc.sync.dma_start(out=selb[:, :], in_=selb_p[:, :])
            sufc = cpool.tile([128, 256], bf16, tag="sufc", name="sufc")
            nc.sync.dma_start(out=sufc[:, :], in_=sufc_p[:, :])

            wqk = []
            wv = []
            xts = []
            for et in range(NE):
                t1 = wpool.tile([128, 512], bf16, tag=f"wqk{et}", name=f"wqk{et}")
                nc.sync.dma_start(out=t1[:, :], in_=wqk_p[et * 128:(et + 1) * 128, :])
                wqk.append(t1)
                t2 = wpool.tile([128, 256], bf16, tag=f"wv{et}", name=f"wv{et}")
                nc.sync.dma_start(out=t2[:, :], in_=wv_p[et * 128:(et + 1) * 128, :])
                wv.append(t2)
                t3 = xpool.tile([128, T], bf16, tag=f"xt{et}", name=f"xt{et}")
                nc.sync.dma_start(out=t3[:, :], in_=xt_p[et * 128:(et + 1) * 128, :])
                xts.append(t3)

            # ---- phase 1b: V projection (natural layout, augmented with ones col) ----
            vaug = []
            vsuf = []

            def emit_v_and_vsuf():
              for tt in range(NT):
                  va = vpool.tile([128, 260], bf16, tag=f"va{tt}", name=f"va{tt}")
                  vaug.append(va)
                  va3 = va[:, :].rearrange("p (h c) -> p h c", c=65)
                  nc.gpsimd.memset(va3[:, :, 64:65], 1.0)
                  vp = pspool.tile([128, 256], fp32, tag="pC", name="vps")
                  for et in range(NE):
                      nc.tensor.matmul(
                          vp[:, :],
                          lhsT=xts[et][:, tt * 128:(tt + 1) * 128],
                          rhs=wv[et][:, :],
                          start=(et == 0), stop=(et == NE - 1),
                      )
                  nc.vector.tensor_copy(
                      va3[:, :, 0:64],
                      vp[:, :].rearrange("p (h d) -> p h d", d=64),
                  )

              # ---- phase 2a: V suffix block sums (all 4 heads per matmul) ----
              vsp = pspool.tile([16, 260], fp32, tag="pD", name="vsps")
              for tt in range(NT):
                  # weighted mask lhsT writes ALL 16 rows every time: no reliance on
                  # overwrite-on-unset-has_written semantics for stale PSUM banks
                  nc.tensor.matmul(
                      vsp[0:16, :],
                      lhsT=sufc[:, tt * 16:(tt + 1) * 16],
                      rhs=vaug[tt][:, 0:260],
                      start=(tt == 0), stop=(tt == NT - 1),
                  )
              vsb4 = vspool.tile([16, 260], bf16, tag="vsuf4", name="vsuf4")
              nc.vector.tensor_copy(vsb4[:, :], vsp[:, :])
              for h in range(HPC):
                  vsuf.append(vsb4[:, h * 65:(h + 1) * 65])

            # ---- phase 1a: QK projection (transposed out) ----
            # psum[grp][tc_] accumulated over E-tiles; grp: 0=Qpair0 1=Kpair0 2=Qpair1 3=Kpair1
            qktiles = []
            for grp in range(4):
                dest = qkpool.tile([128, T], bf16, tag=f"qkt{grp}", name=f"qkt{grp}")
                qktiles.append(dest)
            def emit_qk(grp):
                pss = [pspool.tile([128, 2 * CH], fp32, tag="sAB", name=f"qkps{i}") for i in range(2)]
                for et in range(NE):
                    for tch in range(4):
                        nc.tensor.matmul(
                            pss[tch // 2][:, (tch % 2) * CH:(tch % 2 + 1) * CH],
                            lhsT=wqk[et][:, grp * 128:(grp + 1) * 128],
                            rhs=xts[et][:, tch * CH:(tch + 1) * CH],
                            start=(et == 0), stop=(et == NE - 1),
                        )
                for half in range(2):
                    # evac + bias (per-partition) -> bf16
                    nc.vector.tensor_scalar(
                        out=qktiles[grp][:, half * 2 * CH:(half + 1) * 2 * CH],
                        in0=pss[half][:, :],
                        scalar1=bqk[:, grp:grp + 1],
                        scalar2=None, op0=add,
                    )

            # ---- phase 2b: attention per q-chunk, per head-pair ----
            attnT = []
            for h in range(HPC):
                at = apool.tile([64, T], bf16, tag=f"attnT{h}", name=f"attnT{h}")
                attnT.append(at)
            agin0 = dpool.tile([128, T], bf16, tag="agin0", name="agin0")
            agin1 = dpool.tile([128, T], bf16, tag="agin1", name="agin1")
            agins = [agin0, agin1]
            agout0 = dpool.tile([4 * 128, T], bf16, tag="agout0", name="agout0")
            agout1 = dpool.tile([4 * 128, T], bf16, tag="agout1", name="agout1")

            emit_qk(0)
            emit_qk(1)
            emit_v_and_vsuf()
            for pair in range(2):
                if pair == 1:
                    nc.gpsimd.collective_compute(
                        "AllGather",
                        mybir.AluOpType.bypass,
                        ins=[agin0[:, :].opt()],
                        outs=[agout0[:, :].opt()],
                        replica_groups=[[0, 1, 2, 3], [4, 5, 6, 7]],
                    )
                    emit_qk(2)
                    emit_qk(3)
                qt = qktiles[2 * pair]      # Q pair tile [128, T]
                kt = qktiles[2 * pair + 1]  # K pair tile
                for qc in range(NCH):
                    q0 = qc * CH
                    accs = [pspool.tile([65, CH], fp32, tag=t, name=f"acc{t}") for t in ("pC", "pD")]

                    def do_exp_acc(kj, qoff, n, st2):
                        p2 = ppool.tile([128, 2 * CH], bf16, tag="pAB", name="p2")
                        if n == CH:
                            nc.scalar.activation(p2[:, :], st2[:, :], Exp, scale=0.125)
                        else:
                            nc.scalar.activation(
                                p2[:, :].rearrange("p (s c) -> p s c", s=2)[:, :, 0:n],
                                st2[:, :].rearrange("p (s c) -> p s c", s=2)[:, :, 0:n],
                                Exp, scale=0.125)
                        if kj * 128 >= q0:
                            # diag block: masked entries (k > q) become exp(0)=1, applied
                            # post-exp so the PSUM tile is released by the exp itself
                            nc.vector.copy_predicated(
                                out=p2[:, :].rearrange("p (s c) -> p s c", s=2)[:, :, 0:128],
                                mask=maskc[:, :].rearrange("p (s c) -> p s c", s=1).to_broadcast([128, 2, 128]),
                                data=onesc[:, :].rearrange("p (s c) -> p s c", s=1).to_broadcast([128, 2, 128]),
                            )
                        for s in range(2):
                            h = 2 * pair + s
                            nc.tensor.matmul(
                                accs[s][0:65, qoff - q0:CH],
                                lhsT=vaug[kj][:, h * 65:(h + 1) * 65],
                                rhs=p2[:, s * CH:s * CH + n],
                                start=(kj == 0), stop=False,
                            )

                    prev = None
                    for kj in range(4 * qc + 4):
                        qoff = max(kj * 128, q0)
                        n = q0 + CH - qoff
                        st2 = pspool.tile([128, 2 * CH], fp32, tag="sAB", name="st2")
                        nc.tensor.matmul(
                            st2[:, 0:n],
                            lhsT=kt[0:64, kj * 128:(kj + 1) * 128],
                            rhs=qt[0:64, qoff:q0 + CH],
                            start=True, stop=True, tile_position=(0, 0),
                        )
                        nc.tensor.matmul(
                            st2[:, CH:CH + n],
                            lhsT=kt[64:128, kj * 128:(kj + 1) * 128],
                            rhs=qt[64:128, qoff:q0 + CH],
                            start=True, stop=True, tile_position=(64, 0),
                        )
                        if prev is not None:
                            do_exp_acc(*prev)
                        prev = (kj, qoff, n, st2)
                    do_exp_acc(*prev)
                    # inject suffix sums (P==1 region) + counts via indicator matmul
                    for s in range(2):
                        h = 2 * pair + s
                        nc.tensor.matmul(
                            accs[s][0:65, :],
                            lhsT=vsuf[h],
                            rhs=indc[:, q0:q0 + CH],
                            start=False, stop=True,
                        )
                    # reciprocal directly on Z (partition 64), broadcast via K=1 matmul
                    for s in range(2):
                        h = 2 * pair + s
                        rtmp = rpool.tile([65, CH], bf16, tag="rtmp", name="rtmp")
                        rstf = rpool.tile([65, CH], fp32, tag="rstf", name="rstf")
                        nc.vector.reciprocal(rstf[64:65, :], accs[s][64:65, :])
                        nc.vector.tensor_copy(rtmp[64:65, :], rstf[64:65, :])
                        rb = pspool.tile([64, CH], fp32, tag=("pD" if s == 0 else "pC"), name="rb")
                        nc.tensor.matmul(
                            rb[:, :],
                            lhsT=onesc[64:65, 0:64],
                            rhs=rtmp[64:65, :],
                            start=True, stop=True, tile_position=(64, 0),
                        )
                        rbs = rpool.tile([64, CH], fp32, tag="rbs", name="rbs")
                        nc.vector.tensor_copy(rbs[:, :], rb[:, :])
                        nc.vector.tensor_tensor(
                            out=attnT[h][:, q0:q0 + CH],
                            in0=accs[s][0:64, :], in1=rbs[:, :], op=mult,
                        )
                        nc.sync.dma_start(
                            out=agins[h // 2][(h % 2) * 64:(h % 2 + 1) * 64, q0:q0 + CH],
                            in_=attnT[h][:, q0:q0 + CH],
                        )

            # FFN weights: loaded late so they don't steal DMA bandwidth from xt at startup
            w1sb = []
            w2sb = []
            for et in range(NE):
                t4 = wpool.tile([128, E], bf16, tag=f"w1{et}", name=f"w1{et}")
                nc.sync.dma_start(out=t4[:, :], in_=w1_p[et * 128:(et + 1) * 128, :])
                w1sb.append(t4)
                t5 = wpool.tile([128, E], bf16, tag=f"w2{et}", name=f"w2{et}")
                nc.sync.dma_start(out=t5[:, :], in_=w2_p[et * 128:(et + 1) * 128, :])
                w2sb.append(t5)

            # ---- phase 3: AllGather (head-shard) + indexed gather of this core's t-slice ----
            gix = cpool.tile([128, 8], mybir.dt.int32, tag="gix", name="gix")
            nc.sync.dma_start(out=gix[:, :], in_=gidx_p[:, :])
            nc.gpsimd.collective_compute(
                "AllGather",
                mybir.AluOpType.bypass,
                ins=[agin1[:, :].opt()],
                outs=[agout1[:, :].opt()],
                replica_groups=[[0, 1, 2, 3], [4, 5, 6, 7]],
            )
            # each buffer viewed as [2048, 512]: row R*4 + chunk; this core needs chunk g
            agviews = [agout0[:, :].rearrange("r (c k) -> (r c) k", k=CH),
                       agout1[:, :].rearrange("r (c k) -> (r c) k", k=CH)]
            agt = []
            for et in range(NE):
                t6 = fpool.tile([128, CH], bf16, tag=f"agt{et}", name=f"agt{et}")
                nc.gpsimd.indirect_dma_start(
                    out=t6[:, :],
                    out_offset=None,
                    in_=agviews[et % 2],
                    in_offset=bass.IndirectOffsetOnAxis(ap=gix[:, et:et + 1], axis=0),
                )
                agt.append(t6)

            # ---- phase 4: FFN (transposed): h1T = relu(W1.T @ agt + c1), outT = W2.T @ h1T + b2 ----
            h1t = []
            for e1 in range(NE):
                ps = pspool.tile([128, CH], fp32, tag="pC", name="f1ps")
                # even E-tiles first: they depend only on the first AllGather,
                # so FFN1 starts while the second collective is still in flight
                for idx, et in enumerate((0, 2, 4, 6, 1, 3, 5, 7)):
                    nc.tensor.matmul(
                        ps[:, :],
                        lhsT=w1sb[et][:, e1 * 128:(e1 + 1) * 128],
                        rhs=agt[et][:, :],
                        start=(idx == 0), stop=(idx == NE - 1),
                    )
                ht = fpool.tile([128, CH], bf16, tag=f"h1t{e1}", name=f"h1t{e1}")
                nc.scalar.activation(ht[:, :], ps[:, :], Relu, bias=c1t[:, e1:e1 + 1])
                h1t.append(ht)
            for e2 in range(NE):
                ps = pspool.tile([128, CH], fp32, tag="pD", name="f2ps")
                for et in range(NE):
                    nc.tensor.matmul(
                        ps[:, :],
                        lhsT=w2sb[et][:, e2 * 128:(e2 + 1) * 128],
                        rhs=h1t[et][:, :],
                        start=(et == 0), stop=(et == NE - 1),
                    )
                ot = fpool.tile([128, CH], fp32, tag=f"ot{e2}", name=f"ot{e2}")
                nc.vector.tensor_scalar(
                    out=ot[:, :], in0=ps[:, :],
                    scalar1=b2t[:, e2:e2 + 1], scalar2=None, op0=add,
                )
                nc.sync.dma_start(out=out_p[e2 * 128:(e2 + 1) * 128, :], in_=ot[:, :])

    nc.compile()
    return nc


def _host_prep(x, Wqkv, bqkv, W1, b1, W2, b2):
    """Build the 8 per-core input maps (numpy, host-side layout transforms)."""
    x = np.asarray(x, dtype=np.float32)
    Wqkv = np.asarray(Wqkv, dtype=np.float32)
    bqkv = np.asarray(bqkv, dtype=np.float32)
    W1 = np.asarray(W1, dtype=np.float32)
    b1 = np.asarray(b1, dtype=np.float32)
    W2 = np.asarray(W2, dtype=np.float32)
    b2 = np.asarray(b2, dtype=np.float32)

    # attention-output bias per E index (head-major): bv_full[e] = bqkv[h*192+128+d]
    ei = np.arange(E)
    bv_full = bqkv[(ei // DH) * 3 * DH + 2 * DH + (ei % DH)]
    c1 = W1.T @ bv_full + b1           # (E,)
    c1t = np.ascontiguousarray(c1.reshape(8, 128).T, dtype=np.float32)   # (128, 8)
    b2t = np.ascontiguousarray(b2.reshape(8, 128).T, dtype=np.float32)

    kk, qq = np.meshgrid(np.arange(128), np.arange(128), indexing="ij")
    maskc = (kk > qq).astype(np.uint8)     # complement: 1 where masked   # (128,128)
    onesc = np.ones((128, 128), dtype=BF16)
    jj, tq = np.meshgrid(np.arange(16), np.arange(T), indexing="ij")
    indc = (jj == tq // 128 + 1).astype(BF16)                             # (16, T)

    w1b = W1.astype(BF16)
    w2b = W2.astype(BF16)

    in_maps = []
    for c in range(NCORES):
        b = c // 4
        g = c % 4
        heads = [4 * g + i for i in range(HPC)]
        qcols = lambda h: slice(h * 3 * DH, h * 3 * DH + DH)
        kcols = lambda h: slice(h * 3 * DH + DH, h * 3 * DH + 2 * DH)
        vcols = lambda h: slice(h * 3 * DH + 2 * DH, h * 3 * DH + 3 * DH)
        # wqk groups: [q_h0|q_h1 | k_h0|k_h1 | q_h2|q_h3 | k_h2|k_h3]
        colsel = []
        for pair in range(2):
            h0, h1 = heads[2 * pair], heads[2 * pair + 1]
            colsel += list(range(*qcols(h0).indices(3 * E)))
            colsel += list(range(*qcols(h1).indices(3 * E)))
            colsel += list(range(*kcols(h0).indices(3 * E)))
            colsel += list(range(*kcols(h1).indices(3 * E)))
        # reorder into [Qpair0(128) | Kpair0(128) | Qpair1(128) | Kpair1(128)]
        colsel = np.array(colsel)
        colsel = colsel.reshape(2, 2, 128)[:, [0, 1], :]  # pairs x (q,k) x 128
        colsel = np.concatenate([colsel[0, 0], colsel[0, 1], colsel[1, 0], colsel[1, 1]])
        wqk = np.ascontiguousarray(Wqkv[:, colsel], dtype=BF16)            # (E, 512)
        bqk = np.ascontiguousarray(bqkv[colsel].reshape(4, 128).T, dtype=np.float32)  # (128,4)
        vsel = np.concatenate([np.arange(*vcols(h).indices(3 * E)) for h in heads])
        wv = np.ascontiguousarray(Wqkv[:, vsel], dtype=BF16)               # (E, 256)
        xt = np.ascontiguousarray(x[b].T, dtype=BF16)                      # (E, T)
        pp, ee = np.meshgrid(np.arange(128), np.arange(8), indexing="ij")
        gidx = (((ee // 2) * 128 + pp) * 4 + g).astype(np.int32)           # (128, 8), buf=et%2
        kk4, mm4 = np.meshgrid(np.arange(2), np.arange(128), indexing="ij")
        selb = (kk4 == mm4 // 64).astype(BF16)                             # (2, 128)
        ttj = np.zeros((128, 256), dtype=np.float32)
        for tt in range(16):
            for j in range(16):
                if j <= tt:
                    ttj[:, tt * 16 + j] = 1.0
        sufc = ttj.astype(BF16)                                            # (128, 256)
        in_maps.append({
            "xt": xt, "wqk": wqk, "wv": wv, "bqk": bqk,
            "c1t": c1t, "b2t": b2t, "w1": w1b, "w2": w2b,
            "maskc": maskc, "onesc": onesc, "indc": indc, "gidx": gidx, "selb": selb, "sufc": sufc,
        })
    return in_maps


TRACE = False
LAST_EXEC_NS = None
LAST_RESULTS = None


def kernel(x, Wqkv, bqkv, W1, b1, W2, b2, n_heads=16):
    global LAST_EXEC_NS, LAST_RESULTS
    from concourse import bass_utils

    if "nc" not in _NC_CACHE:
        _NC_CACHE["nc"] = _build_nc()
    nc = _NC_CACHE["nc"]

    in_maps = _host_prep(x, Wqkv, bqkv, W1, b1, W2, b2)
    try:
        res = bass_utils.run_bass_kernel_spmd(
            nc, in_maps, core_ids=list(range(NCORES)), trace=TRACE,
        )
    except ModuleNotFoundError:
        res = bass_utils.run_bass_kernel_spmd(
            nc, in_maps, core_ids=list(range(NCORES)), trace=False,
        )
    LAST_EXEC_NS = res.exec_time_ns
    LAST_RESULTS = res
    y = np.empty((B, T, E), dtype=np.float32)
    for c in range(NCORES):
        b, g = c // 4, c % 4
        outT = np.asarray(res.results[c]["out"], dtype=np.float32)   # (E, TQ)
        y[b, g * TQ:(g + 1) * TQ, :] = outT.T
    return y



# revision 38
# speedup vs baseline: 1.3519x; 1.3519x over previous
"""Distributed Bass kernel for nn_Attention (B=2,T=2048,E=1024,H=16) on 8 trn2 cores.

Sharding: core c = b*4+g handles batch b = c//4, heads 4g..4g+3 for attention
(tensor parallel on qkv columns). After attention, a per-q-chunk 8-core
AllToAll redistributes attention output from head-sharded to t-sharded (each
core receives a 64-t slice of both batches); each core then runs the full FFN
on its t-strips.

Attention math: the reference multiplies raw scores by a lower-triangular 0/1
mask BEFORE softmax (masked logits -> exp(0)=1, not 0). Strictly-above-diagonal
key blocks therefore contribute P==1 exactly: their effect (suffix sums of V
plus a future-key count in the denominator) is injected with one rank-1 matmul
per q-block from precomputed V suffix block sums. Only lower-triangular +
diagonal score blocks are computed/exp'd.

Scheduling: engines execute their streams in emission order, so the emitter
software-pipelines everything around the serial exp stream (the Activation
engine is the scarcest resource): q-chunks are processed in DESCENDING order
(the first chunk needs every V block anyway, so the V/suffix tables are ready
exactly when first needed), Q/K projection is emitted just-in-time per
128-column group x 512-t quarter through a dedicated PSUM staging bank, and V
projection / FFN strips are pumped as filler chains into the PE stream between
score iterations.

Layouts: Q,K live transposed on chip ([qk-dim, t]); V lives natural
([t, v-dim]). The AV product accumulates in natural orientation [q, v-dim]
(output partitions = queries) so softmax normalization is a per-partition
scalar multiply and the attention output is already t-major for the AllToAll.
The softmax denominator accumulates separately in a shared PSUM bank via
ones-column matmuls.
"""

import numpy as np
import ml_dtypes

BF16 = ml_dtypes.bfloat16

B, T, E, H = 2, 2048, 1024, 16
DH = 64          # head dim
HPC = 4          # heads per core
NCORES = 8
NT = T // 128    # 16 t-blocks
NE = E // 128    # 8 E-tiles
CH = 512         # q-chunk size
NCH = T // CH    # 4 chunks
TS = 128         # FFN strip width per chunk (2 batches x 64 t)
ET_ORDER = [1, 3, 5, 7, 0, 2, 4, 6]   # odd xt tiles arrive first (Act queue)

_NC_CACHE = {}


def _build_nc():
    import concourse.bass as bass
    import concourse.mybir as mybir
    import concourse.tile as tile
    from concourse import bacc
    from concourse.bass import BassGpSimd
    from collections import deque

    fp32 = mybir.dt.float32
    bf16 = mybir.dt.bfloat16
    Exp = mybir.ActivationFunctionType.Exp
    add = mybir.AluOpType.add
    mult = mybir.AluOpType.mult
    alu_max = mybir.AluOpType.max

    nc = bacc.Bacc(None, target_bir_lowering=False)

    # ---- parameters (per-core shards staged by host) ----
    xt_p = nc.declare_dram_parameter("xt", [E, T], bf16, isOutput=False)        # x[b].T
    wqkv_p = nc.declare_dram_parameter("wqkv", [E, 768], bf16, isOutput=False)  # [Qp0|Kp0|Qp1|Kp1|V]
    w12_p = nc.declare_dram_parameter("w12", [E, 2 * E], bf16, isOutput=False)  # [W1|W2]
    cb_p = nc.declare_dram_parameter("cb16", [128, 528], bf16, isOutput=False)  # [ones|sufc|cvals|zeros]
    cf_p = nc.declare_dram_parameter("cf32", [128, 20], fp32, isOutput=False)   # [bqk|c1t|b2t]
    mask_p = nc.declare_dram_parameter("maskc", [128, 128], mybir.dt.uint8, isOutput=False)
    out_p = nc.declare_dram_parameter("out", [E, 4 * TS], fp32, isOutput=True)  # 4 strips

    with tile.TileContext(nc) as tc:
        with (
            tc.tile_pool(name="const", bufs=1) as cpool,
            tc.tile_pool(name="wts", bufs=1) as wpool,
            tc.tile_pool(name="xt", bufs=1) as xpool,
            tc.tile_pool(name="qk", bufs=1) as qkpool,
            tc.tile_pool(name="vaug", bufs=1) as vpool,
            tc.tile_pool(name="vsuf", bufs=1) as vspool,
            tc.tile_pool(name="p2", bufs=6) as ppool,
            tc.tile_pool(name="rz", bufs=4) as rzpool,
            tc.tile_pool(name="an", bufs=3) as anpool,
            tc.tile_pool(name="ffn", bufs=4) as fpool,
            tc.tile_pool(name="h1", bufs=2) as hpool,
            tc.tile_pool(name="ot", bufs=2) as opool,
            tc.tile_pool(name="dram", bufs=1, space="DRAM") as dpool,
            tc.tile_pool(name="ps", bufs=2, space="PSUM") as pspool,
            tc.tile_pool(name="pa", bufs=1, space="PSUM") as papool,
        ):
            # ---- constants + W1/W2 on Pool ----
            maskc = cpool.tile([128, 128], mybir.dt.uint8, tag="maskc", name="maskc")
            nc.gpsimd.dma_start(out=maskc[:, :], in_=mask_p[:, :])
            cb16 = cpool.tile([128, 528], bf16, tag="cb16", name="cb16")
            nc.gpsimd.dma_start(out=cb16[:, :], in_=cb_p[:, :])
            cf32 = cpool.tile([128, 20], fp32, tag="cf32", name="cf32")
            nc.gpsimd.dma_start(out=cf32[:, :], in_=cf_p[:, :])
            onesc = cb16[:, 0:128]
            sufc = cb16[:, 128:384]
            cvals = cb16[:, 384:400]
            zrow = cb16[0:1, 400:528]
            bqk = cf32[:, 0:4]
            c1t = cf32[:, 4:12]
            b2t = cf32[:, 12:20]
            w12 = []

            # preload the Exp table off the critical path
            warm = rzpool.tile([1, 1], fp32, tag="warm", name="warm")
            nc.scalar.activation(warm[:, :], cf32[0:1, 0:1], Exp, scale=1.0)

            # ---- wqkv (Q/K cols first) + xt split across SP/Act queues ----
            wqkv = []
            xts = []
            for et in range(NE):
                t1 = wpool.tile([128, 768], bf16, tag=f"wqkv{et}", name=f"wqkv{et}")
                wqkv.append(t1)
                t3 = xpool.tile([128, T], bf16, tag=f"xt{et}", name=f"xt{et}")
                xts.append(t3)
            for et in range(NE):
                nc.sync.dma_start(out=wqkv[et][:, 0:256], in_=wqkv_p[et * 128:(et + 1) * 128, 0:256])
                xq = {0: nc.sync, 1: nc.scalar, 2: nc.gpsimd}[et % 3]
                xq.dma_start(out=xts[et][:, :], in_=xt_p[et * 128:(et + 1) * 128, :])
            for et in range(NE):
                nc.scalar.dma_start(out=wqkv[et][:, 256:768], in_=wqkv_p[et * 128:(et + 1) * 128, 256:768])

            def emit_w12_loads():
                for et in range(NE):
                    t4 = wpool.tile([128, 2 * E], bf16, tag=f"w12{et}", name=f"w12{et}")
                    nc.gpsimd.dma_start(out=t4[:, :], in_=w12_p[et * 128:(et + 1) * 128, :])
                    w12.append(t4)

            qktiles = [qkpool.tile([128, T], bf16, tag=f"qkt{g}", name=f"qkt{g}")
                       for g in range(4)]
            vaug = [None] * NT
            vsb1 = vspool.tile([1, 16 * 256], bf16, tag="vsuf1", name="vsuf1")

            # ---- static PSUM staging banks ----
            # pb: projection/V/vsuf chains, two alternating 256-col regions
            # fz: FFN chains (3 x 128-col regions) + softmax-Z (cols 384..416)
            pb = papool.tile([128, 512], fp32, tag="pb", name="pb")
            fz = papool.tile([128, 512], fp32, tag="fz", name="fz")

            # ---- AllToAll buffers ----
            a2ain = []
            a2aout = []
            for c in range(NCH):
                a2ain.append(dpool.tile([CH, 256], bf16, tag=f"a2i{c}", name=f"a2i{c}"))
                a2aout.append(dpool.tile([CH, 256], bf16, tag=f"a2o{c}", name=f"a2o{c}"))
            REP = [[0, 1, 2, 3, 4, 5, 6, 7]]

            # ================= filler machinery =================
            state = {"pb_reg": 0, "fz_reg": 0, "act_clock": 0.0, "budget": 0.0,
                     "pe_clock": 0.0, "evac": 0}
            fillers = deque()   # (name, cost_ns, gate_ns_or_None, emit_fn)
            stripq = deque()    # FFN strip work, drained after the last scores
            emitted = set()
            a2a_clock = {}

            def pb_region():
                r = state["pb_reg"]
                state["pb_reg"] = (r + 1) % 4
                return pb[:, r * 128:(r + 1) * 128]

            def evac_engine():
                return nc.vector

            def fz_region():
                r = state["fz_reg"]
                state["fz_reg"] = (r + 1) % 3
                return fz[:, r * 128:(r + 1) * 128]

            def mk_proj(grp, tch, h):
                def fn():
                    for q in range(2):
                        ps = pb_region()
                        cols = slice(tch * 512 + h * 256 + q * 128,
                                     tch * 512 + h * 256 + (q + 1) * 128)
                        for i, et in enumerate(ET_ORDER):
                            nc.tensor.matmul(
                                ps, lhsT=wqkv[et][:, grp * 128:(grp + 1) * 128],
                                rhs=xts[et][:, cols],
                                start=(i == 0), stop=(i == NE - 1),
                            )
                        evac_engine().tensor_scalar(
                            out=qktiles[grp][:, cols], in0=ps,
                            scalar1=bqk[:, grp:grp + 1], scalar2=None, op0=add,
                        )
                return (f"g{grp}t{tch}h{h}", 900, None, fn)

            def mk_v(tt):
                def fn():
                    va = vpool.tile([128, 256], bf16, tag=f"va{tt}", name=f"va{tt}")
                    vaug[tt] = va
                    for q in range(2):
                        ps = pb_region()
                        for i, et in enumerate(ET_ORDER):
                            nc.tensor.matmul(
                                ps, lhsT=xts[et][:, tt * 128:(tt + 1) * 128],
                                rhs=wqkv[et][:, 512 + q * 128:512 + (q + 1) * 128],
                                start=(i == 0), stop=(i == NE - 1),
                            )
                        nc.vector.tensor_copy(va[:, q * 128:(q + 1) * 128], ps)
                return (f"V{tt}", 900, None, fn)

            def mk_vsuf():
                def fn():
                    ps = pb[0:16, 0:256]
                    for tt in range(NT):
                        nc.tensor.matmul(
                            ps, lhsT=sufc[:, tt * 16:(tt + 1) * 16],
                            rhs=vaug[tt][:, :],
                            start=(tt == 0), stop=(tt == NT - 1),
                        )
                    vsb4 = vspool.tile([16, 256], bf16, tag="vsuf4", name="vsuf4")
                    nc.vector.tensor_copy(vsb4[:, :], ps)
                    # high rows first: descending chunk order uses them first
                    nc.sync.dma_start(out=vsb1[:, 8 * 256:16 * 256], in_=vsb4[8:16, :])
                    nc.sync.dma_start(out=vsb1[:, 0:8 * 256], in_=vsb4[0:8, :])
                return ("vsuf", 2200, None, fn)

            h1t_by_strip = {}

            def mk_agt(c, gate):
                def fn():
                    agt = []
                    for et in range(NE):
                        t6 = fpool.tile([128, 2 * 64], bf16, tag=f"agt{et}", name=f"agt{c}_{et}")
                        for b in range(2):
                            s = 4 * b + et // 2
                            nc.sync.dma_start_transpose(
                                out=t6[:, b * 64:(b + 1) * 64],
                                in_=a2aout[c][s * 64:(s + 1) * 64,
                                              (et % 2) * 128:(et % 2) * 128 + 128],
                            )
                        agt.append(t6)
                    h1t_by_strip[c] = {"agt": agt, "h1": [], "ot": None}
                return (f"agt{c}", 100, gate, fn)

            def mk_f1(c, e1, gate):
                def fn():
                    st = h1t_by_strip[c]
                    reg = fz_region()
                    for et in range(NE):
                        nc.tensor.matmul(
                            reg, lhsT=w12[et][:, e1 * 128:(e1 + 1) * 128],
                            rhs=st["agt"][et][:, :],
                            start=(et == 0), stop=(et == NE - 1),
                        )
                    ht = hpool.tile([128, TS], bf16, tag=f"h1t{e1}", name=f"h1t{c}_{e1}")
                    nc.vector.tensor_scalar(
                        out=ht[:, :], in0=reg,
                        scalar1=c1t[:, e1:e1 + 1], scalar2=0.0,
                        op0=add, op1=alu_max,
                    )
                    st["h1"].append(ht)
                return (f"f1_{c}_{e1}", 450, gate, fn)

            def mk_f2(c, e2, gate):
                def fn():
                    st = h1t_by_strip[c]
                    if st["ot"] is None:
                        st["ot"] = opool.tile([128, 8 * TS], fp32, tag="ot", name=f"ot{c}")
                    reg = fz_region()
                    for et in range(NE):
                        nc.tensor.matmul(
                            reg, lhsT=w12[et][:, E + e2 * 128:E + (e2 + 1) * 128],
                            rhs=st["h1"][et][:, :],
                            start=(et == 0), stop=(et == NE - 1),
                        )
                    nc.vector.tensor_scalar(
                        out=st["ot"][:, e2 * TS:(e2 + 1) * TS], in0=reg,
                        scalar1=b2t[:, e2:e2 + 1], scalar2=None, op0=add,
                    )
                    if e2 == NE - 1:
                        nc.sync.dma_start(
                            out=out_p[:, c * TS:(c + 1) * TS].rearrange("(et p) t -> p et t", p=128),
                            in_=st["ot"][:, :].rearrange("p (et t) -> p et t", et=8),
                        )
                return (f"f2_{c}_{e2}", 450, gate, fn)

            def push_strip(c):
                mk_agt(c, None)[3]()
                for e1 in range(NE):
                    stripq.append(mk_f1(c, e1, None))
                for e2 in range(NE):
                    stripq.append(mk_f2(c, e2, None))

            def pump_one():
                name, cost, gate, fn = fillers.popleft()
                fn()
                emitted.add(name)
                state["budget"] -= cost
                state["pe_clock"] += cost

            def pump():
                while fillers:
                    name, cost, gate, fn = fillers[0]
                    if gate is not None and state["pe_clock"] < gate:
                        return
                    if state["budget"] < cost:
                        return
                    pump_one()

            def pump_until(name):
                while name not in emitted:
                    assert fillers, f"filler {name} not queued"
                    pump_one()

            # ================= attention =================
            acc_by_chunk = {}
            an_by_chunk = {}

            def z_col(c, idx):
                return 384 + (c % 2) * 16 + idx

            def emit_attn_scores(c, pair):
                q0 = c * CH
                if c not in acc_by_chunk:
                    accs = [papool.tile([128, 2 * 4 * 64], fp32, tag=f"acc{p}",
                                        name=f"acc{c}_{p}") for p in range(2)]
                    acc_by_chunk[c] = [a[:, :].rearrange("p (s qb d) -> p s qb d", s=2, d=64)
                                      for a in accs]
                    # PSUM pending-zero is bank-granular: one start=True zeroing
                    # write per bank per chunk, all accumulation start=False
                    for p in range(2):
                        nc.tensor.matmul(
                            accs[p][:, :], lhsT=zrow, rhs=cb16[0:1, 0:512],
                            start=True, stop=False,
                        )
                    nc.tensor.matmul(
                        fz[:, 384 + (c % 2) * 16:384 + (c % 2) * 16 + 16],
                        lhsT=zrow, rhs=cb16[0:1, 0:16],
                        start=True, stop=False,
                    )
                acc3 = acc_by_chunk[c][pair]
                qt = qktiles[2 * pair]
                kt = qktiles[2 * pair + 1]
                kgrp = 1 if pair == 0 else 3
                qgrp = 0 if pair == 0 else 2

                def do_av(kj, p23, qoff):
                    for s in range(2):
                        h = 2 * pair + s
                        for qbg in range(max(kj, 4 * c), 4 * c + 4):
                            qbl = qbg - 4 * c
                            col = qbg * 128 - qoff
                            is_last = (qbg == NT - 1) and (kj == NT - 1)
                            nc.tensor.matmul(
                                acc3[:, s, qbl, :],
                                lhsT=p23[:, s, col:col + 128],
                                rhs=vaug[kj][:, h * 64:(h + 1) * 64],
                                start=False, stop=is_last,
                            )
                            nc.tensor.matmul(
                                fz[:, z_col(c, h * 4 + qbl):z_col(c, h * 4 + qbl) + 1],
                                lhsT=p23[:, s, col:col + 128],
                                rhs=onesc[:, 0:1],
                                start=False, stop=is_last,
                            )

                pump_until(f"g{qgrp}t{c}h1")
                pending = None
                for kj in range(4 * c + 4):
                    pump_until(f"g{kgrp}t{kj // 4}h1")
                    qoff = max(kj * 128, q0)
                    n = q0 + CH - qoff
                    st2 = pspool.tile([128, 2 * CH], fp32, tag="s", name=f"st{c}_{pair}_{kj}")
                    st3 = st2[:, :].rearrange("p (s q) -> p s q", s=2)
                    nc.tensor.matmul(
                        st2[:, 0:n],
                        lhsT=kt[0:64, kj * 128:(kj + 1) * 128],
                        rhs=qt[0:64, qoff:q0 + CH],
                        start=True, stop=True, tile_position=(0, 0),
                    )
                    nc.tensor.matmul(
                        st2[:, CH:CH + n],
                        lhsT=kt[64:128, kj * 128:(kj + 1) * 128],
                        rhs=qt[64:128, qoff:q0 + CH],
                        start=True, stop=True, tile_position=(64, 0),
                    )
                    p2 = ppool.tile([128, 2 * CH], bf16, tag="p2", name=f"p2_{c}_{pair}_{kj}")
                    p23 = p2[:, :].rearrange("p (s q) -> p s q", s=2)
                    if n == CH:
                        nc.scalar.activation(p2[:, :], st2[:, :], Exp, scale=0.125)
                    else:
                        nc.scalar.activation(p23[:, :, 0:n], st3[:, :, 0:n], Exp, scale=0.125)
                    if kj * 128 >= q0:
                        nc.vector.copy_predicated(
                            out=p23[:, :, 0:128],
                            mask=maskc[:, :].rearrange("p (s c) -> p s c", s=1).to_broadcast([128, 2, 128]),
                            data=onesc[:, :].rearrange("p (s c) -> p s c", s=1).to_broadcast([128, 2, 128]),
                        )
                    exp_est = 2 * n * 0.83 + 217
                    state["act_clock"] += exp_est
                    state["pe_clock"] += 426 + 170
                    state["budget"] = min(state["budget"] + exp_est, 2000)
                    if pending is not None:
                        pump_until(f"V{pending[0]}")
                        do_av(*pending)
                    pending = (kj, p23, qoff)
                    pump()
                pump_until(f"V{pending[0]}")
                do_av(*pending)

            def emit_attn_finish(c, pair):
                acc3 = acc_by_chunk[c][pair]
                pump_until("vsuf")
                for s in range(2):
                    h = 2 * pair + s
                    for qbl in range(4):
                        qbg = 4 * c + qbl
                        if qbg == NT - 1:
                            continue
                        nc.tensor.matmul(
                            acc3[:, s, qbl, :],
                            lhsT=onesc[0:1, 0:128],
                            rhs=vsb1[0:1, (qbg + 1) * 256 + h * 64:(qbg + 1) * 256 + (h + 1) * 64],
                            start=False, stop=True,
                        )
                        nc.tensor.matmul(
                            fz[:, z_col(c, h * 4 + qbl):z_col(c, h * 4 + qbl) + 1],
                            lhsT=onesc[0:1, 0:128],
                            rhs=cvals[0:1, qbg:qbg + 1],
                            start=False, stop=True,
                        )
                rz = rzpool.tile([128, 8], fp32, tag=f"rz{pair}", name=f"rz{c}_{pair}")
                nc.vector.reciprocal(
                    rz[:, :], fz[:, z_col(c, pair * 8):z_col(c, pair * 8) + 8])
                if pair == 0:
                    an_by_chunk[c] = anpool.tile([128, 4 * 256], bf16, tag="an", name=f"an{c}")
                an3 = an_by_chunk[c][:, :].rearrange("p (qb e) -> p qb e", qb=4)
                for s in range(2):
                    h = 2 * pair + s
                    for qbl in range(4):
                        nc.vector.tensor_scalar(
                            out=an3[:, qbl, h * 64:(h + 1) * 64],
                            in0=acc_by_chunk[c][pair][:, s, qbl, :],
                            scalar1=rz[:, s * 4 + qbl:s * 4 + qbl + 1],
                            scalar2=None, op0=mult,
                        )
                nc.sync.dma_start(
                    out=a2ain[c][:, :].rearrange("(qb p) e -> p qb e", qb=4)[:, :, pair * 128:(pair + 1) * 128],
                    in_=an3[:, :, pair * 128:(pair + 1) * 128],
                )

            def emit_a2a(c, eng):
                outap = a2aout[c][:, :]
                eng.collective_compute(
                    "AllToAll",
                    mybir.AluOpType.bypass,
                    ins=[a2ain[c][:, :]],
                    outs=[outap],
                    replica_groups=REP,
                )
                a2a_clock[c] = state["pe_clock"]
                push_strip(c)

            # ================= schedule =================
            # ascending chunks; projection JIT per quarter, V paced for AVs,
            # vsuf forced at chunk 0's finish
            for g in (0, 1):
                fillers.append(mk_proj(g, 0, 0))
                fillers.append(mk_proj(g, 0, 1))
            for tt in range(4):
                fillers.append(mk_v(tt))
            for g in (2, 3):
                fillers.append(mk_proj(g, 0, 0))
                fillers.append(mk_proj(g, 0, 1))
            for tt in range(4, 8):
                fillers.append(mk_v(tt))
            for g in (0, 1):
                fillers.append(mk_proj(g, 1, 0))
                fillers.append(mk_proj(g, 1, 1))
            for tt in range(8, 16):
                fillers.append(mk_v(tt))
            fillers.append(mk_vsuf())
            for g in (2, 3):
                fillers.append(mk_proj(g, 1, 0))
                fillers.append(mk_proj(g, 1, 1))
            for tch in (2, 3):
                for g in (0, 1, 2, 3):
                    fillers.append(mk_proj(g, tch, 0))
                    fillers.append(mk_proj(g, tch, 1))

            a2a_eng = {0: nc.gpsimd, 1: nc.gpsimd, 2: nc.gpsimd, 3: nc.gpsimd}
            for c in (0, 1, 2, 3):
                if c == 1:
                    emit_w12_loads()
                emit_attn_scores(c, 0)
                emit_attn_scores(c, 1)
                emit_attn_finish(c, 0)
                emit_attn_finish(c, 1)
                emit_a2a(c, a2a_eng[c])
                if c == 3:
                    while fillers:
                        pump_one()
                    items = {n: f for n, _, _, f in stripq}
                    stripq.clear()
                    order = []
                    order += [f"f1_0_{j}" for j in range(NE)]
                    order += [f"f1_1_{j}" for j in range(NE)]
                    order += [f"f2_0_{j}" for j in range(NE)]
                    order += [f"f1_2_{j}" for j in range(NE)]
                    order += [f"f2_1_{j}" for j in range(NE)]
                    order += [f"f2_2_{j}" for j in range(NE)]
                    order += [f"f1_3_{j}" for j in range(NE)]
                    order += [f"f2_3_{j}" for j in range(NE)]
                    for n in order:
                        items[n]()

    nc.compile()
    return nc


def _host_prep(x, Wqkv, bqkv, W1, b1, W2, b2):
    """Build the 8 per-core input maps (numpy, host-side layout transforms)."""
    x = np.asarray(x, dtype=np.float32)
    Wqkv = np.asarray(Wqkv, dtype=np.float32)
    bqkv = np.asarray(bqkv, dtype=np.float32)
    W1 = np.asarray(W1, dtype=np.float32)
    b1 = np.asarray(b1, dtype=np.float32)
    W2 = np.asarray(W2, dtype=np.float32)
    b2 = np.asarray(b2, dtype=np.float32)

    # attention-output bias per E index (head-major): bv_full[e] = bqkv[h*192+128+d]
    ei = np.arange(E)
    bv_full = bqkv[(ei // DH) * 3 * DH + 2 * DH + (ei % DH)]
    c1 = W1.T @ bv_full + b1           # (E,)
    c1t = np.ascontiguousarray(c1.reshape(8, 128).T, dtype=np.float32)   # (128, 8)
    b2t = np.ascontiguousarray(b2.reshape(8, 128).T, dtype=np.float32)

    kk, qq = np.meshgrid(np.arange(128), np.arange(128), indexing="ij")
    maskc = (kk > qq).astype(np.uint8)     # complement: 1 where masked   # (128,128)

    cb16 = np.zeros((128, 528), dtype=BF16)
    cb16[:, 0:128] = np.ones((128, 128), dtype=BF16)
    ttj = np.zeros((128, 256), dtype=np.float32)
    for tt in range(16):
        for j in range(16):
            if j <= tt:
                ttj[:, tt * 16 + j] = 1.0
    cb16[:, 128:384] = ttj.astype(BF16)
    cb16[:, 384:400] = (np.arange(15, -1, -1, dtype=np.float32) * 128).astype(BF16)[None, :]

    w12 = np.concatenate([W1, W2], axis=1).astype(BF16)                # (E, 2E)

    in_maps = []
    for c in range(NCORES):
        b = c // 4
        heads = [4 * (c % 4) + i for i in range(HPC)]
        qcols = lambda h: slice(h * 3 * DH, h * 3 * DH + DH)
        kcols = lambda h: slice(h * 3 * DH + DH, h * 3 * DH + 2 * DH)
        vcols = lambda h: slice(h * 3 * DH + 2 * DH, h * 3 * DH + 3 * DH)
        # wqkv groups: [Qpair0(128) | Kpair0(128) | Qpair1(128) | Kpair1(128) | V(256)]
        colsel = []
        for pair in range(2):
            h0, h1 = heads[2 * pair], heads[2 * pair + 1]
            colsel += list(range(*qcols(h0).indices(3 * E)))
            colsel += list(range(*qcols(h1).indices(3 * E)))
            colsel += list(range(*kcols(h0).indices(3 * E)))
            colsel += list(range(*kcols(h1).indices(3 * E)))
        colsel += [i for h in heads for i in range(*vcols(h).indices(3 * E))]
        colsel = np.array(colsel)
        wqkv = np.ascontiguousarray(Wqkv[:, colsel], dtype=BF16)           # (E, 768)
        cf32 = np.zeros((128, 20), dtype=np.float32)
        cf32[:, 0:4] = bqkv[colsel[:512]].reshape(4, 128).T
        cf32[:, 4:12] = c1t
        cf32[:, 12:20] = b2t
        xt = np.ascontiguousarray(x[b].T, dtype=BF16)                      # (E, T)
        in_maps.append({
            "xt": xt, "wqkv": wqkv, "w12": w12,
            "cb16": cb16, "cf32": cf32, "maskc": maskc,
        })
    return in_maps


TRACE = False
LAST_EXEC_NS = None
LAST_RESULTS = None


def kernel(x, Wqkv, bqkv, W1, b1, W2, b2, n_heads=16):
    global LAST_EXEC_NS, LAST_RESULTS
    from concourse import bass_utils

    if "nc" not in _NC_CACHE:
        _NC_CACHE["nc"] = _build_nc()
    nc = _NC_CACHE["nc"]

    in_maps = _host_prep(x, Wqkv, bqkv, W1, b1, W2, b2)
    try:
        res = bass_utils.run_bass_kernel_spmd(
            nc, in_maps, core_ids=list(range(NCORES)), trace=TRACE,
        )
    except ModuleNotFoundError:
        res = bass_utils.run_bass_kernel_spmd(
            nc, in_maps, core_ids=list(range(NCORES)), trace=False,
        )
    LAST_EXEC_NS = res.exec_time_ns
    LAST_RESULTS = res
    y = np.empty((B, T, E), dtype=np.float32)
    for c in range(NCORES):
        outT = np.asarray(res.results[c]["out"], dtype=np.float32)   # (E, 4*2*64)
        for j in range(NCH):
            for b in range(B):
                t0 = CH * j + 64 * c
                y[b, t0:t0 + 64, :] = outT[:, TS * j + 64 * b:TS * j + 64 * b + 64].T
    return y


# revision 42
# speedup vs baseline: 1.5513x; 1.1475x over previous
"""Distributed Bass kernel for nn_Attention (B=2,T=2048,E=1024,H=16) on 8 trn2 cores.

Sharding: core c = b*4+g handles batch b = c//4, heads 4g..4g+3 for attention
(tensor parallel on qkv columns). After attention, a per-q-chunk 8-core
AllToAll redistributes attention output from head-sharded to t-sharded (each
core receives a 64-t slice of both batches); each core then runs the full FFN
on its t-strips.

Attention math: the reference multiplies raw scores by a lower-triangular 0/1
mask BEFORE softmax (masked logits -> exp(0)=1, not 0). Strictly-above-diagonal
key blocks therefore contribute P==1 exactly: their effect (suffix sums of V
plus a future-key count in the denominator) is injected with one rank-1 matmul
per q-block from precomputed V suffix block sums. Only lower-triangular +
diagonal score blocks are computed/exp'd.

Scheduling: engines execute their streams in emission order, so the emitter
software-pipelines everything around the serial exp stream (the Activation
engine is the scarcest resource): q-chunks are processed in DESCENDING order
(the first chunk needs every V block anyway, so the V/suffix tables are ready
exactly when first needed), Q/K projection is emitted just-in-time per
128-column group x 512-t quarter through a dedicated PSUM staging bank, and V
projection / FFN strips are pumped as filler chains into the PE stream between
score iterations.

Layouts: Q,K live transposed on chip ([qk-dim, t]); V lives natural
([t, v-dim]). The AV product accumulates in natural orientation [q, v-dim]
(output partitions = queries) so softmax normalization is a per-partition
scalar multiply and the attention output is already t-major for the AllToAll.
The softmax denominator accumulates separately in a shared PSUM bank via
ones-column matmuls.
"""

import numpy as np
import ml_dtypes

BF16 = ml_dtypes.bfloat16

B, T, E, H = 2, 2048, 1024, 16
DH = 64          # head dim
HPC = 4          # heads per core
NCORES = 8
NT = T // 128    # 16 t-blocks
NE = E // 128    # 8 E-tiles
CH = 512         # q-chunk size
NCH = T // CH    # 4 chunks
TS = 128         # FFN strip width per chunk (2 batches x 64 t)
ET_ORDER = [1, 3, 5, 7, 0, 2, 4, 6]   # odd xt tiles arrive first (Act queue)

_NC_CACHE = {}


def _build_nc():
    import concourse.bass as bass
    import concourse.mybir as mybir
    import concourse.tile as tile
    from concourse import bacc
    from concourse.bass import BassGpSimd
    from collections import deque

    fp32 = mybir.dt.float32
    bf16 = mybir.dt.bfloat16
    Exp = mybir.ActivationFunctionType.Exp
    add = mybir.AluOpType.add
    mult = mybir.AluOpType.mult
    alu_max = mybir.AluOpType.max

    nc = bacc.Bacc(None, target_bir_lowering=False)

    # ---- parameters (per-core shards staged by host) ----
    xt_p = nc.declare_dram_parameter("xt", [E, T], bf16, isOutput=False)        # x[b].T
    wqkv_p = nc.declare_dram_parameter("wqkv", [E, 768], bf16, isOutput=False)  # [Qp0|Kp0|Qp1|Kp1|V]
    w12_p = nc.declare_dram_parameter("w12", [E, 2 * E], bf16, isOutput=False)  # [W1|W2]
    cb_p = nc.declare_dram_parameter("cb16", [128, 528], bf16, isOutput=False)  # [ones|sufc|cvals|zeros]
    cf_p = nc.declare_dram_parameter("cf32", [128, 20], fp32, isOutput=False)   # [bqk|c1t|b2t]
    mask_p = nc.declare_dram_parameter("maskc", [128, 128], mybir.dt.uint8, isOutput=False)
    out_p = nc.declare_dram_parameter("out", [E, 4 * TS], fp32, isOutput=True)  # 4 strips

    with tile.TileContext(nc) as tc:
        with (
            tc.tile_pool(name="const", bufs=1) as cpool,
            tc.tile_pool(name="wts", bufs=1) as wpool,
            tc.tile_pool(name="xt", bufs=1) as xpool,
            tc.tile_pool(name="qk", bufs=1) as qkpool,
            tc.tile_pool(name="vaug", bufs=1) as vpool,
            tc.tile_pool(name="vsuf", bufs=1) as vspool,
            tc.tile_pool(name="p2", bufs=6) as ppool,
            tc.tile_pool(name="rz", bufs=4) as rzpool,
            tc.tile_pool(name="an", bufs=3) as anpool,
            tc.tile_pool(name="ffn", bufs=4) as fpool,
            tc.tile_pool(name="h1", bufs=2) as hpool,
            tc.tile_pool(name="ot", bufs=2) as opool,
            tc.tile_pool(name="dram", bufs=1, space="DRAM") as dpool,
            tc.tile_pool(name="ps", bufs=2, space="PSUM") as pspool,
            tc.tile_pool(name="pa", bufs=1, space="PSUM") as papool,
        ):
            # ---- constants + W1/W2 on Pool ----
            maskc = cpool.tile([128, 128], mybir.dt.uint8, tag="maskc", name="maskc")
            nc.gpsimd.dma_start(out=maskc[:, :], in_=mask_p[:, :])
            cb16 = cpool.tile([128, 528], bf16, tag="cb16", name="cb16")
            nc.gpsimd.dma_start(out=cb16[:, :], in_=cb_p[:, :])
            cf32 = cpool.tile([128, 20], fp32, tag="cf32", name="cf32")
            nc.gpsimd.dma_start(out=cf32[:, :], in_=cf_p[:, :])
            onesc = cb16[:, 0:128]
            sufc = cb16[:, 128:384]
            cvals = cb16[:, 384:400]
            zrow = cb16[0:1, 400:528]
            bqk = cf32[:, 0:4]
            c1t = cf32[:, 4:12]
            b2t = cf32[:, 12:20]
            w12 = []

            # preload the Exp table off the critical path
            warm = rzpool.tile([1, 1], fp32, tag="warm", name="warm")
            nc.scalar.activation(warm[:, :], cf32[0:1, 0:1], Exp, scale=1.0)

            # ---- wqkv (Q/K cols first) + xt split across SP/Act queues ----
            wqkv = []
            xts = []
            for et in range(NE):
                t1 = wpool.tile([128, 768], bf16, tag=f"wqkv{et}", name=f"wqkv{et}")
                wqkv.append(t1)
                t3 = xpool.tile([128, T], bf16, tag=f"xt{et}", name=f"xt{et}")
                xts.append(t3)
            for et in range(NE):
                nc.sync.dma_start(out=wqkv[et][:, 0:256], in_=wqkv_p[et * 128:(et + 1) * 128, 0:256])
            # xt loaded in 512-col quarters, tch-major, spread over 3 queues:
            # the first projection chains only need the t0 quarters
            qs = [nc.sync, nc.scalar, nc.gpsimd]
            i = 0
            for tch in range(4):
                for et in range(NE):
                    qs[i % 3].dma_start(
                        out=xts[et][:, tch * 512:(tch + 1) * 512],
                        in_=xt_p[et * 128:(et + 1) * 128, tch * 512:(tch + 1) * 512])
                    i += 1
                if tch == 0:
                    for et in range(NE):
                        nc.scalar.dma_start(out=wqkv[et][:, 256:768],
                                            in_=wqkv_p[et * 128:(et + 1) * 128, 256:768])

            def emit_w12_loads():
                for et in range(NE):
                    t4 = wpool.tile([128, 2 * E], bf16, tag=f"w12{et}", name=f"w12{et}")
                    nc.gpsimd.dma_start(out=t4[:, :], in_=w12_p[et * 128:(et + 1) * 128, :])
                    w12.append(t4)

            qktiles = [qkpool.tile([128, T], bf16, tag=f"qkt{g}", name=f"qkt{g}")
                       for g in range(4)]
            vaug = [None] * NT
            vsb1 = vspool.tile([1, 16 * 256], bf16, tag="vsuf1", name="vsuf1")

            # ---- static PSUM staging banks ----
            # pb: projection/V/vsuf chains, two alternating 256-col regions
            # fz: FFN chains (3 x 128-col regions) + softmax-Z (cols 384..416)
            pb = papool.tile([128, 512], fp32, tag="pb", name="pb")
            fz = papool.tile([128, 512], fp32, tag="fz", name="fz")

            # ---- AllToAll buffers ----
            a2ain = []
            a2aout = []
            for c in range(NCH):
                a2ain.append(dpool.tile([CH, 256], bf16, tag=f"a2i{c}", name=f"a2i{c}"))
                a2aout.append(dpool.tile([CH, 256], bf16, tag=f"a2o{c}", name=f"a2o{c}"))
            REP = [[0, 1, 2, 3, 4, 5, 6, 7]]

            # ================= filler machinery =================
            state = {"pb_reg": 0, "fz_reg": 0, "act_clock": 0.0, "budget": 0.0,
                     "pe_clock": 0.0, "evac": 0}
            fillers = deque()   # (name, cost_ns, gate_ns_or_None, emit_fn)
            stripq = deque()    # FFN strip work, drained after the last scores
            emitted = set()
            a2a_clock = {}

            def pb_region():
                r = state["pb_reg"]
                state["pb_reg"] = (r + 1) % 4
                return pb[:, r * 128:(r + 1) * 128]

            def evac_engine():
                return nc.vector

            def fz_region():
                r = state["fz_reg"]
                state["fz_reg"] = (r + 1) % 3
                return fz[:, r * 128:(r + 1) * 128]

            strip_regions = []

            def strip_region():
                r = state["fz_reg"]
                if strip_regions:
                    state["fz_reg"] = (r + 1) % len(strip_regions)
                    return strip_regions[r % len(strip_regions)]
                return fz_region()

            def mk_proj(grp, tch, h):
                def fn():
                    for q in range(2):
                        ps = pb_region()
                        cols = slice(tch * 512 + h * 256 + q * 128,
                                     tch * 512 + h * 256 + (q + 1) * 128)
                        for i, et in enumerate(ET_ORDER):
                            nc.tensor.matmul(
                                ps, lhsT=wqkv[et][:, grp * 128:(grp + 1) * 128],
                                rhs=xts[et][:, cols],
                                start=(i == 0), stop=(i == NE - 1),
                            )
                        evac_engine().tensor_scalar(
                            out=qktiles[grp][:, cols], in0=ps,
                            scalar1=bqk[:, grp:grp + 1], scalar2=None, op0=add,
                        )
                return (f"g{grp}t{tch}h{h}", 900, None, fn)

            def mk_v(tt):
                def fn():
                    va = vpool.tile([128, 256], bf16, tag=f"va{tt}", name=f"va{tt}")
                    vaug[tt] = va
                    for q in range(2):
                        ps = pb_region()
                        for i, et in enumerate(ET_ORDER):
                            nc.tensor.matmul(
                                ps, lhsT=xts[et][:, tt * 128:(tt + 1) * 128],
                                rhs=wqkv[et][:, 512 + q * 128:512 + (q + 1) * 128],
                                start=(i == 0), stop=(i == NE - 1),
                            )
                        nc.vector.tensor_copy(va[:, q * 128:(q + 1) * 128], ps)
                return (f"V{tt}", 900, None, fn)

            def mk_vsuf():
                def fn():
                    ps = pb[0:16, 0:256]
                    for tt in range(NT):
                        nc.tensor.matmul(
                            ps, lhsT=sufc[:, tt * 16:(tt + 1) * 16],
                            rhs=vaug[tt][:, :],
                            start=(tt == 0), stop=(tt == NT - 1),
                        )
                    vsb4 = vspool.tile([16, 256], bf16, tag="vsuf4", name="vsuf4")
                    nc.vector.tensor_copy(vsb4[:, :], ps)
                    # high rows first: descending chunk order uses them first
                    nc.sync.dma_start(out=vsb1[:, 8 * 256:16 * 256], in_=vsb4[8:16, :])
                    nc.sync.dma_start(out=vsb1[:, 0:8 * 256], in_=vsb4[0:8, :])
                return ("vsuf", 2200, None, fn)

            h1t_by_strip = {}

            def mk_agt(c, gate):
                def fn():
                    agt = []
                    for et in range(NE):
                        t6 = fpool.tile([128, 2 * 64], bf16, tag=f"agt{et}", name=f"agt{c}_{et}")
                        for b in range(2):
                            s = 4 * b + et // 2
                            nc.sync.dma_start_transpose(
                                out=t6[:, b * 64:(b + 1) * 64],
                                in_=a2aout[c][s * 64:(s + 1) * 64,
                                              (et % 2) * 128:(et % 2) * 128 + 128],
                            )
                        agt.append(t6)
                    h1t_by_strip[c] = {"agt": agt, "h1": [], "ot": None}
                return (f"agt{c}", 100, gate, fn)

            def mk_f1(c, e1, gate):
                def fn():
                    st = h1t_by_strip[c]
                    reg = strip_region()
                    for et in range(NE):
                        nc.tensor.matmul(
                            reg, lhsT=w12[et][:, e1 * 128:(e1 + 1) * 128],
                            rhs=st["agt"][et][:, :],
                            start=(et == 0), stop=(et == NE - 1),
                        )
                    ht = hpool.tile([128, TS], bf16, tag=f"h1t{e1}", name=f"h1t{c}_{e1}")
                    nc.vector.tensor_scalar(
                        out=ht[:, :], in0=reg,
                        scalar1=c1t[:, e1:e1 + 1], scalar2=0.0,
                        op0=add, op1=alu_max,
                    )
                    st["h1"].append(ht)
                return (f"f1_{c}_{e1}", 450, gate, fn)

            def mk_f2(c, e2, gate):
                def fn():
                    st = h1t_by_strip[c]
                    if st["ot"] is None:
                        st["ot"] = opool.tile([128, 8 * TS], fp32, tag="ot", name=f"ot{c}")
                    reg = strip_region()
                    for et in range(NE):
                        nc.tensor.matmul(
                            reg, lhsT=w12[et][:, E + e2 * 128:E + (e2 + 1) * 128],
                            rhs=st["h1"][et][:, :],
                            start=(et == 0), stop=(et == NE - 1),
                        )
                    nc.vector.tensor_scalar(
                        out=st["ot"][:, e2 * TS:(e2 + 1) * TS], in0=reg,
                        scalar1=b2t[:, e2:e2 + 1], scalar2=None, op0=add,
                    )
                    if e2 == NE - 1:
                        nc.sync.dma_start(
                            out=out_p[:, c * TS:(c + 1) * TS].rearrange("(et p) t -> p et t", p=128),
                            in_=st["ot"][:, :].rearrange("p (et t) -> p et t", et=8),
                        )
                return (f"f2_{c}_{e2}", 450, gate, fn)

            def push_strip(c):
                mk_agt(c, None)[3]()
                for e1 in range(NE):
                    stripq.append(mk_f1(c, e1, None))
                for e2 in range(NE):
                    stripq.append(mk_f2(c, e2, None))

            def pump_one():
                name, cost, gate, fn = fillers.popleft()
                fn()
                emitted.add(name)
                state["budget"] -= cost
                state["pe_clock"] += cost

            def pump():
                while fillers:
                    name, cost, gate, fn = fillers[0]
                    if gate is not None and state["pe_clock"] < gate:
                        return
                    if state["budget"] < cost:
                        return
                    pump_one()

            def pump_n(n):
                for _ in range(n):
                    if fillers:
                        pump_one()

            def pump_until(name):
                while name not in emitted:
                    assert fillers, f"filler {name} not queued"
                    pump_one()

            # ================= attention =================
            acc_by_chunk = {}
            acc_raw = {}
            an_by_chunk = {}

            def z_col(c, idx):
                return 384 + (c % 2) * 16 + idx

            def emit_attn_scores(c, pair):
                q0 = c * CH
                if c not in acc_by_chunk:
                    accs = [papool.tile([128, 2 * 4 * 64], fp32, tag=f"acc{p}",
                                        name=f"acc{c}_{p}") for p in range(2)]
                    acc_by_chunk[c] = [a[:, :].rearrange("p (s qb d) -> p s qb d", s=2, d=64)
                                      for a in accs]
                    acc_raw[c] = [a[:, :] for a in accs]
                    # PSUM pending-zero is bank-granular: one start=True zeroing
                    # write per bank per chunk, all accumulation start=False
                    for p in range(2):
                        nc.tensor.matmul(
                            accs[p][:, :], lhsT=zrow, rhs=cb16[0:1, 0:512],
                            start=True, stop=False,
                        )
                    nc.tensor.matmul(
                        fz[:, 384 + (c % 2) * 16:384 + (c % 2) * 16 + 16],
                        lhsT=zrow, rhs=cb16[0:1, 0:16],
                        start=True, stop=False,
                    )
                acc3 = acc_by_chunk[c][pair]
                qt = qktiles[2 * pair]
                kt = qktiles[2 * pair + 1]
                kgrp = 1 if pair == 0 else 3
                qgrp = 0 if pair == 0 else 2

                def do_av(kj, p23, qoff):
                    for s in range(2):
                        h = 2 * pair + s
                        for qbg in range(max(kj, 4 * c), 4 * c + 4):
                            qbl = qbg - 4 * c
                            col = qbg * 128 - qoff
                            is_last = (qbg == NT - 1) and (kj == NT - 1)
                            nc.tensor.matmul(
                                acc3[:, s, qbl, :],
                                lhsT=p23[:, s, col:col + 128],
                                rhs=vaug[kj][:, h * 64:(h + 1) * 64],
                                start=False, stop=is_last,
                            )
                            nc.tensor.matmul(
                                fz[:, z_col(c, h * 4 + qbl):z_col(c, h * 4 + qbl) + 1],
                                lhsT=p23[:, s, col:col + 128],
                                rhs=onesc[:, 0:1],
                                start=False, stop=is_last,
                            )

                pump_until(f"g{qgrp}t{c}h1")
                pending = None
                for kj in range(4 * c + 4):
                    pump_until(f"g{kgrp}t{kj // 4}h1")
                    qoff = max(kj * 128, q0)
                    n = q0 + CH - qoff
                    st2 = pspool.tile([128, 2 * CH], fp32, tag="s", name=f"st{c}_{pair}_{kj}")
                    st3 = st2[:, :].rearrange("p (s q) -> p s q", s=2)
                    nc.tensor.matmul(
                        st2[:, 0:n],
                        lhsT=kt[0:64, kj * 128:(kj + 1) * 128],
                        rhs=qt[0:64, qoff:q0 + CH],
                        start=True, stop=True, tile_position=(0, 0),
                    )
                    nc.tensor.matmul(
                        st2[:, CH:CH + n],
                        lhsT=kt[64:128, kj * 128:(kj + 1) * 128],
                        rhs=qt[64:128, qoff:q0 + CH],
                        start=True, stop=True, tile_position=(64, 0),
                    )
                    p2 = ppool.tile([128, 2 * CH], bf16, tag="p2", name=f"p2_{c}_{pair}_{kj}")
                    p23 = p2[:, :].rearrange("p (s q) -> p s q", s=2)
                    if n == CH:
                        nc.scalar.activation(p2[:, :], st2[:, :], Exp, scale=0.125)
                    else:
                        nc.scalar.activation(p23[:, :, 0:n], st3[:, :, 0:n], Exp, scale=0.125)
                    if kj * 128 >= q0:
                        nc.vector.copy_predicated(
                            out=p23[:, :, 0:128],
                            mask=maskc[:, :].rearrange("p (s c) -> p s c", s=1).to_broadcast([128, 2, 128]),
                            data=onesc[:, :].rearrange("p (s c) -> p s c", s=1).to_broadcast([128, 2, 128]),
                        )
                    exp_est = 2 * n * 0.83 + 217
                    state["act_clock"] += exp_est
                    state["pe_clock"] += 426 + 170
                    if pending is not None:
                        pump_until(f"V{pending[0]}")
                        do_av(*pending)
                    pending = (kj, p23, qoff)
                    pump_n(pump_plan.get((c, pair, kj), 0))
                pump_until(f"V{pending[0]}")
                do_av(*pending)

            def emit_attn_finish(c, pair):
                acc3 = acc_by_chunk[c][pair]
                pump_until("vsuf")
                for s in range(2):
                    h = 2 * pair + s
                    for qbl in range(4):
                        qbg = 4 * c + qbl
                        if qbg == NT - 1:
                            continue
                        nc.tensor.matmul(
                            acc3[:, s, qbl, :],
                            lhsT=onesc[0:1, 0:128],
                            rhs=vsb1[0:1, (qbg + 1) * 256 + h * 64:(qbg + 1) * 256 + (h + 1) * 64],
                            start=False, stop=True,
                        )
                        nc.tensor.matmul(
                            fz[:, z_col(c, h * 4 + qbl):z_col(c, h * 4 + qbl) + 1],
                            lhsT=onesc[0:1, 0:128],
                            rhs=cvals[0:1, qbg:qbg + 1],
                            start=False, stop=True,
                        )
                rz = rzpool.tile([128, 8], fp32, tag=f"rz{pair}", name=f"rz{c}_{pair}")
                nc.vector.reciprocal(
                    rz[:, :], fz[:, z_col(c, pair * 8):z_col(c, pair * 8) + 8])
                if pair == 0:
                    an_by_chunk[c] = anpool.tile([128, 4 * 256], bf16, tag="an", name=f"an{c}")
                an3 = an_by_chunk[c][:, :].rearrange("p (qb e) -> p qb e", qb=4)
                for s in range(2):
                    h = 2 * pair + s
                    for qbl in range(4):
                        nc.vector.tensor_scalar(
                            out=an3[:, qbl, h * 64:(h + 1) * 64],
                            in0=acc_by_chunk[c][pair][:, s, qbl, :],
                            scalar1=rz[:, s * 4 + qbl:s * 4 + qbl + 1],
                            scalar2=None, op0=mult,
                        )
                nc.sync.dma_start(
                    out=a2ain[c][:, :].rearrange("(qb p) e -> p qb e", qb=4)[:, :, pair * 128:(pair + 1) * 128],
                    in_=an3[:, :, pair * 128:(pair + 1) * 128],
                )

            def emit_a2a(c, eng):
                outap = a2aout[c][:, :]
                eng.collective_compute(
                    "AllToAll",
                    mybir.AluOpType.bypass,
                    ins=[a2ain[c][:, :]],
                    outs=[outap],
                    replica_groups=REP,
                )
                a2a_clock[c] = state["pe_clock"]
                push_strip(c)

            # ================= schedule =================
            # ascending chunks; projection JIT per quarter; all V blocks and
            # vsuf pushed as early as possible so chunk 0's finish (and with it
            # the serialized Pool A2A chain) fires early
            for g in (0, 1):
                fillers.append(mk_proj(g, 0, 0))
                fillers.append(mk_proj(g, 0, 1))
            for tt in range(4):
                fillers.append(mk_v(tt))
            for g in (2, 3):
                fillers.append(mk_proj(g, 0, 0))
                fillers.append(mk_proj(g, 0, 1))
            for tt in range(4, 16):
                fillers.append(mk_v(tt))
            fillers.append(mk_vsuf())
            for g in (0, 1, 2, 3):
                fillers.append(mk_proj(g, 1, 0))
                fillers.append(mk_proj(g, 1, 1))
            for tch in (2, 3):
                for g in (0, 1, 2, 3):
                    fillers.append(mk_proj(g, tch, 0))
                    fillers.append(mk_proj(g, tch, 1))

            # (chunk, pair, kj) -> number of filler chains to pump after that
            # score iteration: spread the 12us of V/vsuf over the c0 windows
            pump_plan = {}
            for kj in range(4):
                pump_plan[(0, 0, kj)] = 2
                pump_plan[(0, 1, kj)] = 2
            for kj in range(8):
                pump_plan[(1, 0, kj)] = 1
                pump_plan[(1, 1, kj)] = 1
            for kj in range(12):
                pump_plan[(2, 0, kj)] = 1
                pump_plan[(2, 1, kj)] = 1

            a2a_eng = {0: nc.gpsimd, 1: nc.gpsimd, 2: nc.gpsimd, 3: nc.gpsimd}
            for c in (0, 1, 2, 3):
                if c == 1:
                    emit_w12_loads()
                emit_attn_scores(c, 0)
                emit_attn_finish(c, 0)
                emit_attn_scores(c, 1)
                emit_attn_finish(c, 1)
                emit_a2a(c, a2a_eng[c])
                if c == 3:
                    while fillers:
                        pump_one()
                    for p in range(2):
                        for r in range(4):
                            strip_regions.append(
                                acc_by_chunk[3][p].tensor_handle[:, r * 128:(r + 1) * 128]
                                if hasattr(acc_by_chunk[3][p], 'tensor_handle') else None)
                    strip_regions.clear()
                    for p in range(2):
                        raw = acc_raw[3][p]
                        for r in range(4):
                            strip_regions.append(raw[:, r * 128:(r + 1) * 128])
                    state["fz_reg"] = 0
                    items = {n: f for n, _, _, f in stripq}
                    stripq.clear()
                    order = []
                    order += [f"f1_0_{j}" for j in range(NE)]
                    order += [f"f1_1_{j}" for j in range(NE)]
                    order += [f"f2_0_{j}" for j in range(NE)]
                    order += [f"f1_2_{j}" for j in range(NE)]
                    order += [f"f2_1_{j}" for j in range(NE)]
                    # f2 of strip 2 deliberately last among the ready strips:
                    # it bridges the PE gap while A2A_3 is still in flight
                    order += [f"f2_2_{j}" for j in range(NE)]
                    order += [f"f1_3_{j}" for j in range(NE)]
                    order += [f"f2_3_{j}" for j in range(NE)]
                    for n in order:
                        items[n]()

    nc.compile()
    return nc


def _host_prep(x, Wqkv, bqkv, W1, b1, W2, b2):
    """Build the 8 per-core input maps (numpy, host-side layout transforms)."""
    x = np.asarray(x, dtype=np.float32)
    Wqkv = np.asarray(Wqkv, dtype=np.float32)
    bqkv = np.asarray(bqkv, dtype=np.float32)
    W1 = np.asarray(W1, dtype=np.float32)
    b1 = np.asarray(b1, dtype=np.float32)
    W2 = np.asarray(W2, dtype=np.float32)
    b2 = np.asarray(b2, dtype=np.float32)

    # attention-output bias per E index (head-major): bv_full[e] = bqkv[h*192+128+d]
    ei = np.arange(E)
    bv_full = bqkv[(ei // DH) * 3 * DH + 2 * DH + (ei % DH)]
    c1 = W1.T @ bv_full + b1           # (E,)
    c1t = np.ascontiguousarray(c1.reshape(8, 128).T, dtype=np.float32)   # (128, 8)
    b2t = np.ascontiguousarray(b2.reshape(8, 128).T, dtype=np.float32)

    kk, qq = np.meshgrid(np.arange(128), np.arange(128), indexing="ij")
    maskc = (kk > qq).astype(np.uint8)     # complement: 1 where masked   # (128,128)

    cb16 = np.zeros((128, 528), dtype=BF16)
    cb16[:, 0:128] = np.ones((128, 128), dtype=BF16)
    ttj = np.zeros((128, 256), dtype=np.float32)
    for tt in range(16):
        for j in range(16):
            if j <= tt:
                ttj[:, tt * 16 + j] = 1.0
    cb16[:, 128:384] = ttj.astype(BF16)
    cb16[:, 384:400] = (np.arange(15, -1, -1, dtype=np.float32) * 128).astype(BF16)[None, :]

    w12 = np.concatenate([W1, W2], axis=1).astype(BF16)                # (E, 2E)

    in_maps = []
    for c in range(NCORES):
        b = c // 4
        heads = [4 * (c % 4) + i for i in range(HPC)]
        qcols = lambda h: slice(h * 3 * DH, h * 3 * DH + DH)
        kcols = lambda h: slice(h * 3 * DH + DH, h * 3 * DH + 2 * DH)
        vcols = lambda h: slice(h * 3 * DH + 2 * DH, h * 3 * DH + 3 * DH)
        # wqkv groups: [Qpair0(128) | Kpair0(128) | Qpair1(128) | Kpair1(128) | V(256)]
        colsel = []
        for pair in range(2):
            h0, h1 = heads[2 * pair], heads[2 * pair + 1]
            colsel += list(range(*qcols(h0).indices(3 * E)))
            colsel += list(range(*qcols(h1).indices(3 * E)))
            colsel += list(range(*kcols(h0).indices(3 * E)))
            colsel += list(range(*kcols(h1).indices(3 * E)))
        colsel += [i for h in heads for i in range(*vcols(h).indices(3 * E))]
        colsel = np.array(colsel)
        wqkv = np.ascontiguousarray(Wqkv[:, colsel], dtype=BF16)           # (E, 768)
        cf32 = np.zeros((128, 20), dtype=np.float32)
        cf32[:, 0:4] = bqkv[colsel[:512]].reshape(4, 128).T
        cf32[:, 4:12] = c1t
        cf32[:, 12:20] = b2t
        xt = np.ascontiguousarray(x[b].T, dtype=BF16)                      # (E, T)
        in_maps.append({
            "xt": xt, "wqkv": wqkv, "w12": w12,
            "cb16": cb16, "cf32": cf32, "maskc": maskc,
        })
    return in_maps


TRACE = False
LAST_EXEC_NS = None
LAST_RESULTS = None


def kernel(x, Wqkv, bqkv, W1, b1, W2, b2, n_heads=16):
    global LAST_EXEC_NS, LAST_RESULTS
    from concourse import bass_utils

    if "nc" not in _NC_CACHE:
        _NC_CACHE["nc"] = _build_nc()
    nc = _NC_CACHE["nc"]

    in_maps = _host_prep(x, Wqkv, bqkv, W1, b1, W2, b2)
    try:
        res = bass_utils.run_bass_kernel_spmd(
            nc, in_maps, core_ids=list(range(NCORES)), trace=TRACE,
        )
    except ModuleNotFoundError:
        res = bass_utils.run_bass_kernel_spmd(
            nc, in_maps, core_ids=list(range(NCORES)), trace=False,
        )
    LAST_EXEC_NS = res.exec_time_ns
    LAST_RESULTS = res
    y = np.empty((B, T, E), dtype=np.float32)
    for c in range(NCORES):
        outT = np.asarray(res.results[c]["out"], dtype=np.float32)   # (E, 4*2*64)
        for j in range(NCH):
            for b in range(B):
                t0 = CH * j + 64 * c
                y[b, t0:t0 + 64, :] = outT[:, TS * j + 64 * b:TS * j + 64 * b + 64].T
    return y
